# revision 12
# baseline (speedup 1.0000x reference)
"""BertSelfAttention on 8 Trainium2 NeuronCores.

Problem: B=2, S=2048, H=1024, 16 heads x 64. Sharding: batch x head-group
(2 batches x 4 head-groups of 4 heads = 8 cores). Each core computes
q/k/v projections for its 4 heads and full attention over them.

Schedule (v4): the kernel is PE-bound overall (attention matmuls 109us +
projections 41us vs 141us of softmax exp on ACT), and the TRN2 tensor
engine clock ramps with sustained load (matmuls run ~2x slower until the
PE has been continuously busy), so the whole design keeps the PE dense
from t~=1us:
  - x is DMA'd in s-major chunks ([128, 8h, 128s] slices of a single
    [128, 8, 2048] tile) split across the SP and ACT hw DGE queues, so
    the first projection chains fire ~2us in while later chunks stream
  - a dozen no-op identity matmuls at t=0 start the clock ramp
  - pre-attention computes only v0/q0/k0-chunk0 chains; every other
    projection chain (v1..v15, k0 tail, q0 tail, q1/k1) is emitted as a
    filler inside the attention kc-loops, keeping the PE saturated while
    the ACT exp stream drains; v[j] is always emitted >=1 iteration
    before the PV matmul that consumes it
  - projection psum chains time-share the two score psum slots via tag
    rotation; PSUM = 2 score slots (4KB) + 2 pv accumulators (4KB)
  - block tails (pv->sbuf copy, fp16 PE transpose, normalize, store)
    are deferred past the next block's first iterations
All matmuls fp16 (fp8 DoubleRow measured no faster than fp16 on this
HW and its quantization noise blows the 2e-2 max-err budget).
"""

import sys

sys.path.insert(0, "/opt/trn_rl_repo")

import numpy as np

import concourse.bass as bass
import concourse.tile as tile
from concourse.masks import make_identity
from concourse import bacc, mybir
from concourse.bass_utils import run_bass_kernel_spmd

F32 = mybir.dt.float32
F16 = mybir.dt.float16
EXP = mybir.ActivationFunctionType.Exp

B, S, H = 2, 2048, 1024
NH, HD = 16, 64
G = 4                 # head-groups (cores per batch)
NHL = NH // G         # heads per core
O = NHL * HD          # 256 output features per core
IC = H // 128         # 8 contraction chunks
KC = S // 128         # 16 key chunks
QCHUNK = 1024
NQ = S // QCHUNK
NEG = -1.0e30


def build_nc():
    nc = bacc.Bacc(None, target_bir_lowering=False)
    xq_d = [nc.declare_dram_parameter(f"xq{i}", [128, IC, 512], F16,
                                      isOutput=False) for i in range(4)]
    wqT = nc.declare_dram_parameter("wqT", [128, IC, O], F16, isOutput=False)
    wkT = nc.declare_dram_parameter("wkT", [128, IC, O], F16, isOutput=False)
    wvT = nc.declare_dram_parameter("wvT", [128, IC, O], F16, isOutput=False)
    bqk = nc.declare_dram_parameter("bqk", [128, 4], F32, isOutput=False)
    bvb = nc.declare_dram_parameter("bvb", [128, NHL * (HD + 1)], F16,
                                    isOutput=False)
    mb = nc.declare_dram_parameter("mb", [128, KC], F32, isOutput=False)
    out = nc.declare_dram_parameter("out", [S, O], F32, isOutput=True)

    with tile.TileContext(nc) as tc:
        with tc.tile_pool(name="consts", bufs=1) as consts, \
             tc.tile_pool(name="persist", bufs=1) as persist, \
             tc.tile_pool(name="scps", bufs=1, space="PSUM") as scps, \
             tc.tile_pool(name="pvps", bufs=1, space="PSUM") as pvps, \
             tc.tile_pool(name="pbp", bufs=3) as pbp, \
             tc.tile_pool(name="tailp", bufs=2) as tailp:
            ident = consts.tile([128, 128], F16, tag="ident", name="ident")
            make_identity(nc, ident)
            mb_sb = consts.tile([128, KC], F32, tag="mb", name="mb_sb")
            bqk_sb = consts.tile([128, 4], F32, tag="bqk", name="bqk_sb")
            bvb_sb = consts.tile([128, NHL * (HD + 1)], F16, tag="bvb",
                                 name="bvb_sb")

            xt = [persist.tile([128, IC, 512], F16, tag=f"xt{i}",
                               name=f"xt{i}") for i in range(4)]
            wq = persist.tile([128, IC, O], F16, tag="wq", name="wqs")
            wk = persist.tile([128, IC, O], F16, tag="wk", name="wks")
            wv = persist.tile([128, IC, O], F16, tag="wv", name="wvs")
            qT = [persist.tile([128, S], F16, tag=f"qT{i}", name=f"qT{i}")
                  for i in range(2)]
            kT = [persist.tile([128, S], F16, tag=f"kT{i}", name=f"kT{i}")
                  for i in range(2)]
            vS = [persist.tile([128, NHL * (HD + 1)], F16, tag=f"v{i}",
                               name=f"v{i}") for i in range(KC)]

            # ACT-queue side: exp-table warmup first, then consts + odd
            # x s-chunks; SP side: weights then even x s-chunks. The two
            # hw DGE queues stream concurrently.
            dummy = consts.tile([128, 1], F32, tag="dummy", name="dummy")
            nc.vector.memset(dummy, 0.0)
            nc.scalar.activation(dummy, dummy, EXP)
            nc.scalar.dma_start(out=mb_sb, in_=mb[:, :])
            nc.scalar.dma_start(out=bqk_sb, in_=bqk[:, :])
            nc.scalar.dma_start(out=bvb_sb, in_=bvb[:, :])
            nc.sync.dma_start(out=wq, in_=wqT[:, :, :])
            nc.scalar.dma_start(out=wv, in_=wvT[:, :, :])
            nc.sync.dma_start(out=wk, in_=wkT[:, :, :])
            nc.scalar.dma_start(out=xt[1], in_=xq_d[1][:, :, :])
            nc.sync.dma_start(out=xt[0], in_=xq_d[0][:, :, :])
            nc.scalar.dma_start(out=xt[3], in_=xq_d[3][:, :, :])
            nc.sync.dma_start(out=xt[2], in_=xq_d[2][:, :, :])

            # clock-ramp warmup: dependency-free identity matmuls
            for w in range(12):
                wps = scps.tile([128, 128], F32, tag=f"sc{w % 2}",
                                name=f"warm{w}")
                nc.tensor.matmul(wps, lhsT=ident, rhs=ident,
                                 start=True, stop=True)

            sc_n = [0]

            def proj_qk(wt, ob, dest, bcol, sc):
                ps = scps.tile([128, 512], F32, tag=f"sc{sc_n[0] % 2}",
                               name=f"pqk{bcol}_{sc}")
                sc_n[0] += 1
                for i in range(IC):
                    nc.tensor.matmul(
                        ps,
                        lhsT=wt[:, i, ob * 128:(ob + 1) * 128],
                        rhs=xt[sc][:, i, :],
                        start=(i == 0), stop=(i == IC - 1))
                nc.vector.tensor_scalar_add(
                    dest[:, sc * 512:(sc + 1) * 512], ps,
                    bqk_sb[:, bcol:bcol + 1])

            bvview = bvb_sb.rearrange("p (h d) -> p h d", h=NHL)

            def proj_v(sc):
                ps = scps.tile([128, O], F32, tag=f"sc{sc_n[0] % 2}",
                               name=f"pv{sc}")
                sc_n[0] += 1
                qt4, r4 = sc // 4, sc % 4
                for i in range(IC):
                    nc.tensor.matmul(
                        ps,
                        lhsT=xt[qt4][:, i, r4 * 128:(r4 + 1) * 128],
                        rhs=wv[:, i, :],
                        start=(i == 0), stop=(i == IC - 1))
                vview = vS[sc].rearrange("p (h d) -> p h d", h=NHL)
                nc.vector.tensor_add(
                    vview[:, :, 0:HD],
                    ps.rearrange("p (h d) -> p h d", h=NHL),
                    bvview[:, :, 0:HD])
                nc.vector.tensor_copy(
                    vview[:, :, HD:HD + 1], bvview[:, :, HD:HD + 1])

            # ---- pre-attention: the bare minimum for block 0 iter 0
            proj_v(0)
            proj_qk(wq, 0, qT[0], 0, 0)
            proj_qk(wk, 0, kT[0], 2, 0)
            proj_qk(wq, 0, qT[0], 0, 1)

            # fillers: (block, iter) -> list of chains. v[j] is always
            # emitted at an iter < j (its PV matmul must never wait on a
            # later PE instruction); k0 chunk c covers kc 4c..4c+3.
            fill = {
                (0, 0): [lambda: proj_v(1), lambda: proj_v(2)],
                (0, 1): [lambda: proj_v(3),
                         lambda: proj_qk(wk, 0, kT[0], 2, 1)],
                (0, 2): [lambda: proj_v(4)],
                (0, 3): [lambda: proj_v(5), lambda: proj_v(6)],
                (0, 4): [lambda: proj_qk(wk, 0, kT[0], 2, 2)],
                (0, 5): [lambda: proj_v(7), lambda: proj_v(8)],
                (0, 6): [lambda: proj_v(9)],
                (0, 7): [lambda: proj_qk(wk, 0, kT[0], 2, 3),
                         lambda: proj_v(10)],
                (0, 8): [lambda: proj_v(11)],
                (0, 9): [lambda: proj_v(12)],
                (0, 10): [lambda: proj_v(13)],
                (0, 11): [lambda: proj_v(14)],
                (0, 12): [lambda: proj_v(15)],
                (0, 13): [lambda: proj_qk(wq, 0, qT[0], 0, 2)],
                (0, 14): [lambda: proj_qk(wq, 0, qT[0], 0, 3)],
            }
            for sc, it in enumerate((1, 3, 5, 7)):
                fill[(1, it)] = [lambda sc=sc: proj_qk(wq, 1, qT[1], 1, sc)]
            for sc, it in enumerate((9, 11, 13, 14)):
                fill[(1, it)] = [lambda sc=sc: proj_qk(wk, 1, kT[1], 3, sc)]

            # ---- attention blocks
            blocks = [(0, 0), (0, 1), (1, 0), (1, 1)]
            pending_tail = [None]

            def tail_a(pv, bi):
                ovt = []
                for e in range(2):
                    o_t = tailp.tile([HD + 1, QCHUNK], F16, tag=f"ovt{e}",
                                     name=f"ovt{e}_{bi}")
                    nc.vector.tensor_copy(o_t, pv[e])
                    ovt.append(o_t)
                return ovt

            def tail_b(hp, qc, ovt):
                nj = QCHUNK // 128
                for e in range(2):
                    hh = 2 * hp + e
                    tr = scps.tile([128, nj, 128], F16, tag=f"sc{e}",
                                   name=f"tr{e}_{hp}{qc}")
                    for jb in range(nj):
                        nc.tensor.transpose(
                            tr[:, jb, 0:HD + 1],
                            ovt[e][:, jb * 128:(jb + 1) * 128],
                            ident[0:HD + 1, 0:HD + 1])
                    rc = tailp.tile([128, nj], F32, tag=f"rc{e}",
                                    name=f"rc{e}_{hp}{qc}")
                    nc.vector.reciprocal(rc, tr[:, :, HD])
                    osb = tailp.tile([128, nj * HD], F32, tag=f"osb{e}",
                                     name=f"osb{e}_{hp}{qc}")
                    for jb in range(nj):
                        nc.vector.tensor_scalar_mul(
                            osb[:, jb * HD:(jb + 1) * HD],
                            tr[:, jb, 0:HD], rc[:, jb:jb + 1])
                    dst = out[qc * QCHUNK:(qc + 1) * QCHUNK,
                              hh * HD:(hh + 1) * HD]
                    dst = dst.rearrange("(j p) d -> p j d", p=128)
                    nc.sync.dma_start(
                        out=dst,
                        in_=osb.rearrange("p (j d) -> p j d", j=nj))

            for bi, (hp, qc) in enumerate(blocks):
                pv = [pvps.tile([HD + 1, QCHUNK], F32, tag=f"pv{e}",
                                name=f"pv{e}_{bi}") for e in range(2)]
                for kc in range(KC):
                    ts = []
                    for e in range(2):
                        t = scps.tile([128, QCHUNK], F32, tag=f"sc{e}",
                                      name=f"ts{e}_{bi}_{kc}")
                        lo = e * 64
                        for n in range(QCHUNK // 512):
                            nc.tensor.matmul(
                                t[:, n * 512:(n + 1) * 512],
                                lhsT=kT[hp][lo:lo + 64,
                                            kc * 128:(kc + 1) * 128],
                                rhs=qT[hp][lo:lo + 64,
                                           qc * QCHUNK + n * 512:
                                           qc * QCHUNK + (n + 1) * 512],
                                start=True, stop=True)
                        ts.append(t)
                    pbs = []
                    for e in range(2):
                        pb = pbp.tile([128, QCHUNK], F16, tag=f"pb{e}",
                                      name=f"pb{e}_{bi}_{kc}")
                        nc.scalar.activation(
                            pb, ts[e], EXP,
                            bias=mb_sb[:, kc:kc + 1], scale=0.125)
                        pbs.append(pb)
                    for e in range(2):
                        hh = 2 * hp + e
                        for n in range(QCHUNK // 512):
                            nc.tensor.matmul(
                                pv[e][:, n * 512:(n + 1) * 512],
                                lhsT=vS[kc][:, hh * 65:hh * 65 + 65],
                                rhs=pbs[e][:, n * 512:(n + 1) * 512],
                                start=(kc == 0), stop=(kc == KC - 1))
                    if kc == 2 and pending_tail[0] is not None:
                        tail_b(*pending_tail[0])
                        pending_tail[0] = None
                    for f in fill.pop((bi, kc), ()):
                        f()
                ovt = tail_a(pv, bi)
                pending_tail[0] = (hp, qc, ovt)
            tail_b(*pending_tail[0])
    nc.finalize()
    return nc


_NC_CACHE = None


def _get_nc():
    global _NC_CACHE
    if _NC_CACHE is None:
        _NC_CACHE = build_nc()
    return _NC_CACHE


def _hmaj(a):
    # [1024, N] -> [128, 8, N]: out[p, c, n] = a[128c + p, n]
    n = a.shape[1]
    return np.ascontiguousarray(
        a.reshape(IC, 128, n).transpose(1, 0, 2)).astype(np.float16)


def make_in_maps(inputs, attention_mask, Wq, bq, Wk, bk, Wv, bv):
    x = np.asarray(inputs, dtype=np.float32)
    mask = np.asarray(attention_mask)
    Wq = np.asarray(Wq, dtype=np.float32)
    Wk = np.asarray(Wk, dtype=np.float32)
    Wv = np.asarray(Wv, dtype=np.float32)
    bq = np.asarray(bq, dtype=np.float32)
    bk = np.asarray(bk, dtype=np.float32)
    bv = np.asarray(bv, dtype=np.float32)

    xqb = [[_hmaj(np.ascontiguousarray(x[b].T[:, qt * 512:(qt + 1) * 512]))
            for qt in range(4)] for b in range(B)]
    mbb = [np.ascontiguousarray(
        ((1.0 - mask[b].astype(np.float32)) * NEG).reshape(KC, 128).T)
        for b in range(B)]
    in_maps = []
    for c in range(8):
        b, g = c // G, c % G
        cols = slice(g * O, (g + 1) * O)
        bqs, bks = bq[cols], bk[cols]
        bvc = np.concatenate(
            [np.concatenate([bv[cols][h * 64:(h + 1) * 64], [1.0]])
             for h in range(NHL)]).astype(np.float32)
        bvbc = np.ascontiguousarray(
            np.broadcast_to(bvc[None, :], (128, len(bvc))))
        im = {f"xq{qt}": xqb[b][qt] for qt in range(4)}
        in_maps.append({
            **im,
            "wqT": _hmaj(np.ascontiguousarray(Wq.T[:, cols])),
            "wkT": _hmaj(np.ascontiguousarray(Wk.T[:, cols])),
            "wvT": _hmaj(np.ascontiguousarray(Wv.T[:, cols])),
            "bqk": np.ascontiguousarray(
                np.stack([bqs[:128], bqs[128:], bks[:128], bks[128:]],
                         axis=1)),
            "bvb": bvbc.astype(np.float16),
            "mb": mbb[b],
        })
    return in_maps


def assemble(results):
    outs = [results[c]["out"] for c in range(8)]
    full = np.stack(
        [np.concatenate(outs[b * G:(b + 1) * G], axis=1) for b in range(B)])
    return np.ascontiguousarray(full.astype(np.float32))


def kernel(**inputs) -> np.ndarray:
    nc = _get_nc()
    in_maps = make_in_maps(**inputs)
    res = run_bass_kernel_spmd(nc, in_maps, core_ids=list(range(8)))
    return assemble(res.results)


# revision 15
# speedup vs baseline: 1.3653x; 1.3653x over previous
"""BertSelfAttention on 8 Trainium2 NeuronCores.

Problem: B=2, S=2048, H=1024, 16 heads x 64. Sharding: batch x head-group
(2 batches x 4 head-groups of 4 heads = 8 cores). Each core computes
q/k/v projections for its 4 heads and full attention over them.

Schedule (v6): ACT must stream 128 exps x ~1.1us back-to-back while the
PE fits attention matmuls (110us) plus all projections (41us) around it,
and the chip halves the tensor clock after ~150us of dense PE activity —
so the design minimizes total PE-busy and keeps the exp stream dense:
  - q-blocks of 512: per kc one merged [128,1024] score tile (both
    heads; exp bias only depends on key position), ONE exp, 2 score +
    2 pv matmuls; psum = 2 score slots (kc-parity) + 2 pv accumulators
    + a dedicated 2-bank slot for projection chains, so projection
    fillers never sit on the score WAR chain
  - V is projected in transposed orientation (vT[d, s], N=512 matmuls
    like q/k) and moved to the [key, d] layout the PV matmul needs by
    XBAR DMA transposes (64-row slabs) — off the PE entirely; the
    denominator ones-column is memset once
  - x is DMA'd as four contiguous s-quarter tensors split across the
    SP and ACT DGE queues; only q0/k0/vT0 chunk-0 chains run before
    attention starts (~10us), every other projection chain is a filler
    inside the attention loops with explicit deadlines
  - block tails (pv copy, fp16 PE transpose, normalize, store) are
    deferred past the next block's second iteration
"""

import sys

sys.path.insert(0, "/opt/trn_rl_repo")

import numpy as np

import concourse.bass as bass
import concourse.tile as tile
from concourse.masks import make_identity
from concourse import bacc, mybir
from concourse.bass_utils import run_bass_kernel_spmd

F32 = mybir.dt.float32
F16 = mybir.dt.float16
EXP = mybir.ActivationFunctionType.Exp

B, S, H = 2, 2048, 1024
NH, HD = 16, 64
G = 4                 # head-groups (cores per batch)
NHL = NH // G         # heads per core
O = NHL * HD          # 256 output features per core
IC = H // 128         # 8 contraction chunks
KC = S // 128         # 16 key chunks
QB = 512              # q block size
NQ = S // QB          # 4 q blocks per head pair
NEG = -1.0e30


def build_nc():
    nc = bacc.Bacc(None, target_bir_lowering=False)
    xq_d = [nc.declare_dram_parameter(f"xq{i}", [128, IC, 512], F16,
                                      isOutput=False) for i in range(4)]
    wqT = nc.declare_dram_parameter("wqT", [128, IC, O], F16, isOutput=False)
    wkT = nc.declare_dram_parameter("wkT", [128, IC, O], F16, isOutput=False)
    wvT = nc.declare_dram_parameter("wvT", [128, IC, O], F16, isOutput=False)
    bqk = nc.declare_dram_parameter("bqk", [128, 4], F32, isOutput=False)
    bv2 = nc.declare_dram_parameter("bv2", [128, 2], F32, isOutput=False)
    mb = nc.declare_dram_parameter("mb", [128, KC], F32, isOutput=False)
    out = nc.declare_dram_parameter("out", [S, O], F32, isOutput=True)

    with tile.TileContext(nc) as tc:
        with tc.tile_pool(name="consts", bufs=1) as consts, \
             tc.tile_pool(name="persist", bufs=1) as persist, \
             tc.tile_pool(name="scp", bufs=1, space="PSUM") as scp, \
             tc.tile_pool(name="pvp", bufs=1, space="PSUM") as pvp, \
             tc.tile_pool(name="fps", bufs=2, space="PSUM") as fps, \
             tc.tile_pool(name="pbp", bufs=2) as pbp, \
             tc.tile_pool(name="ttmp", bufs=2) as ttmp, \
             tc.tile_pool(name="tailp", bufs=2) as tailp:
            ident = consts.tile([128, 128], F16, tag="ident", name="ident")
            make_identity(nc, ident)
            mb_sb = consts.tile([128, KC], F32, tag="mb", name="mb_sb")
            bqk_sb = consts.tile([128, 4], F32, tag="bqk", name="bqk_sb")
            bv2_sb = consts.tile([128, 2], F32, tag="bv2", name="bv2_sb")

            xt = [persist.tile([128, IC, 512], F16, tag=f"xt{i}",
                               name=f"xt{i}") for i in range(4)]
            wq = persist.tile([128, IC, O], F16, tag="wq", name="wqs")
            wk = persist.tile([128, IC, O], F16, tag="wk", name="wks")
            wv = persist.tile([128, IC, O], F16, tag="wv", name="wvs")
            qT = [persist.tile([128, S], F16, tag=f"qT{i}", name=f"qT{i}")
                  for i in range(2)]
            kT = [persist.tile([128, S], F16, tag=f"kT{i}", name=f"kT{i}")
                  for i in range(2)]
            vT = [persist.tile([128, S], F16, tag=f"vT{i}", name=f"vT{i}")
                  for i in range(2)]
            # [key-in-chunk, kc, head-half, d + denominator-ones]
            vS = [persist.tile([128, KC, 2, HD + 1], F16, tag=f"vS{i}",
                               name=f"vS{i}") for i in range(2)]
            for ob in range(2):
                nc.vector.memset(vS[ob][:, :, :, HD:HD + 1], 1.0)

            # ACT-queue: exp-table warmup, consts, v weights, odd x
            # quarters; SP-queue: q/k weights, even x quarters.
            dummy = consts.tile([128, 1], F32, tag="dummy", name="dummy")
            nc.vector.memset(dummy, 0.0)
            nc.scalar.activation(dummy, dummy, EXP)
            nc.scalar.dma_start(out=mb_sb, in_=mb[:, :])
            nc.scalar.dma_start(out=bqk_sb, in_=bqk[:, :])
            nc.scalar.dma_start(out=bv2_sb, in_=bv2[:, :])
            nc.scalar.dma_start(out=wv, in_=wvT[:, :, :])
            nc.sync.dma_start(out=wq, in_=wqT[:, :, :])
            nc.sync.dma_start(out=wk, in_=wkT[:, :, :])
            nc.sync.dma_start(out=xt[0], in_=xq_d[0][:, :, :])
            nc.scalar.dma_start(out=xt[1], in_=xq_d[1][:, :, :])
            nc.sync.dma_start(out=xt[2], in_=xq_d[2][:, :, :])
            nc.scalar.dma_start(out=xt[3], in_=xq_d[3][:, :, :])

            def proj_qk(wt, ob, dest, bcol, sc):
                ps = fps.tile([128, 512], F32, tag="fp",
                              name=f"pqk{bcol}_{sc}")
                for i in range(IC):
                    nc.tensor.matmul(
                        ps,
                        lhsT=wt[:, i, ob * 128:(ob + 1) * 128],
                        rhs=xt[sc][:, i, :],
                        start=(i == 0), stop=(i == IC - 1))
                nc.vector.tensor_scalar_add(
                    dest[:, sc * 512:(sc + 1) * 512], ps,
                    bqk_sb[:, bcol:bcol + 1])

            def proj_vt(ob, sc):
                ps = fps.tile([128, 512], F32, tag="fp", name=f"pvt{ob}_{sc}")
                for i in range(IC):
                    nc.tensor.matmul(
                        ps,
                        lhsT=wv[:, i, ob * 128:(ob + 1) * 128],
                        rhs=xt[sc][:, i, :],
                        start=(i == 0), stop=(i == IC - 1))
                nc.vector.tensor_scalar_add(
                    vT[ob][:, sc * 512:(sc + 1) * 512], ps,
                    bv2_sb[:, ob:ob + 1])
                for eh in range(2):
                    # tt[p, kc, d] = vT[64*eh + d, 512*sc + 128*kc + p];
                    # XBAR lands in a plain contiguous tile (HW mishandles
                    # scattered out APs), DVE scatters into vS
                    tt = ttmp.tile([128, 4, HD], F16, tag="tt",
                                   name=f"tt{ob}_{sc}_{eh}")
                    nc.sync.dma_start_transpose(
                        out=tt,
                        in_=vT[ob][eh * 64:(eh + 1) * 64,
                                   sc * 512:(sc + 1) * 512])
                    nc.vector.tensor_copy(
                        vS[ob][:, 4 * sc:4 * sc + 4, eh, 0:HD], tt)

            # ---- pre-attention: the bare minimum for block 0 iter 0
            proj_qk(wq, 0, qT[0], 0, 0)
            proj_qk(wk, 0, kT[0], 2, 0)
            proj_vt(0, 0)

            fill = {}
            fill[(0, 0)] = lambda: proj_qk(wk, 0, kT[0], 2, 1)
            fill[(0, 1)] = lambda: proj_vt(0, 1)
            fill[(0, 3)] = lambda: proj_qk(wk, 0, kT[0], 2, 2)
            fill[(0, 5)] = lambda: proj_vt(0, 2)
            fill[(0, 7)] = lambda: proj_qk(wk, 0, kT[0], 2, 3)
            fill[(0, 9)] = lambda: proj_vt(0, 3)
            fill[(0, 11)] = lambda: proj_qk(wq, 0, qT[0], 0, 1)
            fill[(0, 13)] = lambda: proj_vt(1, 0)
            fill[(0, 15)] = lambda: proj_qk(wq, 1, qT[1], 1, 0)
            fill[(1, 1)] = lambda: proj_qk(wq, 0, qT[0], 0, 2)
            fill[(1, 3)] = lambda: proj_vt(1, 1)
            fill[(1, 5)] = lambda: proj_qk(wk, 1, kT[1], 3, 0)
            fill[(1, 7)] = lambda: proj_qk(wq, 0, qT[0], 0, 3)
            fill[(1, 9)] = lambda: proj_vt(1, 2)
            fill[(1, 11)] = lambda: proj_qk(wk, 1, kT[1], 3, 1)
            fill[(1, 13)] = lambda: proj_qk(wq, 1, qT[1], 1, 1)
            fill[(2, 1)] = lambda: proj_vt(1, 3)
            fill[(2, 3)] = lambda: proj_qk(wk, 1, kT[1], 3, 2)
            fill[(2, 5)] = lambda: proj_qk(wq, 1, qT[1], 1, 2)
            fill[(2, 7)] = lambda: proj_qk(wk, 1, kT[1], 3, 3)
            fill[(2, 9)] = lambda: proj_qk(wq, 1, qT[1], 1, 3)

            # ---- attention blocks
            blocks = [(hp, qc) for hp in range(2) for qc in range(NQ)]
            pending_tail = [None]

            def tail_a(pv, bi):
                ovt = []
                for e in range(2):
                    o_t = tailp.tile([HD + 1, QB], F16, tag=f"ovt{e}",
                                     name=f"ovt{e}_{bi}")
                    nc.vector.tensor_copy(o_t, pv[e])
                    ovt.append(o_t)
                return ovt

            def tail_b(hp, qc, ovt):
                nj = QB // 128
                for e in range(2):
                    hh = 2 * hp + e
                    tr = scp.tile([128, nj, 128], F16, tag=f"sc{e}",
                                  name=f"tr{e}_{hp}{qc}")
                    for jb in range(nj):
                        nc.tensor.transpose(
                            tr[:, jb, 0:HD + 1],
                            ovt[e][:, jb * 128:(jb + 1) * 128],
                            ident[0:HD + 1, 0:HD + 1])
                    rc = tailp.tile([128, nj], F32, tag=f"rc{e}",
                                    name=f"rc{e}_{hp}{qc}")
                    nc.vector.reciprocal(rc, tr[:, :, HD])
                    osb = tailp.tile([128, nj * HD], F32, tag=f"osb{e}",
                                     name=f"osb{e}_{hp}{qc}")
                    for jb in range(nj):
                        nc.vector.tensor_scalar_mul(
                            osb[:, jb * HD:(jb + 1) * HD],
                            tr[:, jb, 0:HD], rc[:, jb:jb + 1])
                    dst = out[qc * QB:(qc + 1) * QB, hh * HD:(hh + 1) * HD]
                    dst = dst.rearrange("(j p) d -> p j d", p=128)
                    nc.sync.dma_start(
                        out=dst,
                        in_=osb.rearrange("p (j d) -> p j d", j=nj))

            for bi, (hp, qc) in enumerate(blocks):
                pv = [pvp.tile([HD + 1, QB], F32, tag=f"pv{e}",
                               name=f"pv{e}_{bi}") for e in range(2)]
                for kc in range(KC):
                    t = scp.tile([128, 2 * QB], F32, tag=f"sc{kc % 2}",
                                 name=f"ts_{bi}_{kc}")
                    for e in range(2):
                        nc.tensor.matmul(
                            t[:, e * QB:(e + 1) * QB],
                            lhsT=kT[hp][e * 64:e * 64 + 64,
                                        kc * 128:(kc + 1) * 128],
                            rhs=qT[hp][e * 64:e * 64 + 64,
                                       qc * QB:(qc + 1) * QB],
                            start=True, stop=True)
                    pb = pbp.tile([128, 2 * QB], F16, tag=f"pb{kc % 2}",
                                  name=f"pb_{bi}_{kc}")
                    nc.scalar.activation(
                        pb, t, EXP, bias=mb_sb[:, kc:kc + 1], scale=0.125)
                    for e in range(2):
                        nc.tensor.matmul(
                            pv[e],
                            lhsT=vS[hp][:, kc, e, :],
                            rhs=pb[:, e * QB:(e + 1) * QB],
                            start=(kc == 0), stop=(kc == KC - 1))
                    if kc == 2 and pending_tail[0] is not None:
                        tail_b(*pending_tail[0])
                        pending_tail[0] = None
                    f = fill.pop((bi, kc), None)
                    if f is not None:
                        f()
                ovt = tail_a(pv, bi)
                pending_tail[0] = (hp, qc, ovt)
            tail_b(*pending_tail[0])
    nc.finalize()
    return nc


_NC_CACHE = None


def _get_nc():
    global _NC_CACHE
    if _NC_CACHE is None:
        _NC_CACHE = build_nc()
    return _NC_CACHE


def _hmaj(a):
    # [1024, N] -> [128, 8, N]: out[p, c, n] = a[128c + p, n]
    n = a.shape[1]
    return np.ascontiguousarray(
        a.reshape(IC, 128, n).transpose(1, 0, 2)).astype(np.float16)


def make_in_maps(inputs, attention_mask, Wq, bq, Wk, bk, Wv, bv):
    x = np.asarray(inputs, dtype=np.float32)
    mask = np.asarray(attention_mask)
    Wq = np.asarray(Wq, dtype=np.float32)
    Wk = np.asarray(Wk, dtype=np.float32)
    Wv = np.asarray(Wv, dtype=np.float32)
    bq = np.asarray(bq, dtype=np.float32)
    bk = np.asarray(bk, dtype=np.float32)
    bv = np.asarray(bv, dtype=np.float32)

    xqb = [[_hmaj(np.ascontiguousarray(x[b].T[:, qt * 512:(qt + 1) * 512]))
            for qt in range(4)] for b in range(B)]
    mbb = [np.ascontiguousarray(
        ((1.0 - mask[b].astype(np.float32)) * NEG).reshape(KC, 128).T)
        for b in range(B)]
    in_maps = []
    for c in range(8):
        b, g = c // G, c % G
        cols = slice(g * O, (g + 1) * O)
        bqs, bks = bq[cols], bk[cols]
        bvs = bv[cols]
        # per-partition v bias for the vT orientation: partition p of
        # vT[ob] is head 2*ob + p//64, feature p%64
        bv2c = np.stack(
            [bvs[ob * 128:(ob + 1) * 128] for ob in range(2)], axis=1)
        im = {f"xq{qt}": xqb[b][qt] for qt in range(4)}
        in_maps.append({
            **im,
            "wqT": _hmaj(np.ascontiguousarray(Wq.T[:, cols])),
            "wkT": _hmaj(np.ascontiguousarray(Wk.T[:, cols])),
            "wvT": _hmaj(np.ascontiguousarray(Wv.T[:, cols])),
            "bqk": np.ascontiguousarray(
                np.stack([bqs[:128], bqs[128:], bks[:128], bks[128:]],
                         axis=1)),
            "bv2": np.ascontiguousarray(bv2c),
            "mb": mbb[b],
        })
    return in_maps


def assemble(results):
    outs = [results[c]["out"] for c in range(8)]
    full = np.stack(
        [np.concatenate(outs[b * G:(b + 1) * G], axis=1) for b in range(B)])
    return np.ascontiguousarray(full.astype(np.float32))


def kernel(**inputs) -> np.ndarray:
    nc = _get_nc()
    in_maps = make_in_maps(**inputs)
    res = run_bass_kernel_spmd(nc, in_maps, core_ids=list(range(8)))
    return assemble(res.results)


# revision 17
# speedup vs baseline: 1.3918x; 1.0194x over previous
"""BertSelfAttention on 8 Trainium2 NeuronCores.

Problem: B=2, S=2048, H=1024, 16 heads x 64. Sharding: batch x head-group
(2 batches x 4 head-groups of 4 heads = 8 cores). Each core computes
q/k/v projections for its 4 heads and full attention over them.

Schedule (v6): ACT must stream 128 exps x ~1.1us back-to-back while the
PE fits attention matmuls (110us) plus all projections (41us) around it,
and the chip halves the tensor clock after ~150us of dense PE activity —
so the design minimizes total PE-busy and keeps the exp stream dense:
  - q-blocks of 512: per kc one merged [128,1024] score tile (both
    heads; exp bias only depends on key position), ONE exp, 2 score +
    2 pv matmuls; psum = 2 score slots (kc-parity) + 2 pv accumulators
    + a dedicated 2-bank slot for projection chains, so projection
    fillers never sit on the score WAR chain
  - V is projected in transposed orientation (vT[d, s], N=512 matmuls
    like q/k) and moved to the [key, d] layout the PV matmul needs by
    XBAR DMA transposes (64-row slabs) — off the PE entirely; the
    denominator ones-column is memset once
  - x is DMA'd as four contiguous s-quarter tensors split across the
    SP and ACT DGE queues; only q0/k0/vT0 chunk-0 chains run before
    attention starts (~10us), every other projection chain is a filler
    inside the attention loops with explicit deadlines
  - block tails (pv copy, fp16 PE transpose, normalize, store) are
    deferred past the next block's second iteration
"""

import sys

sys.path.insert(0, "/opt/trn_rl_repo")

import numpy as np

import concourse.bass as bass
import concourse.tile as tile
from concourse.masks import make_identity
from concourse import bacc, mybir
from concourse.bass_utils import run_bass_kernel_spmd

F32 = mybir.dt.float32
F16 = mybir.dt.float16
EXP = mybir.ActivationFunctionType.Exp

B, S, H = 2, 2048, 1024
NH, HD = 16, 64
G = 4                 # head-groups (cores per batch)
NHL = NH // G         # heads per core
O = NHL * HD          # 256 output features per core
IC = H // 128         # 8 contraction chunks
KC = S // 128         # 16 key chunks
QB = 512              # q block size
NQ = S // QB          # 4 q blocks per head pair
NEG = -1.0e30


def build_nc():
    nc = bacc.Bacc(None, target_bir_lowering=False)
    xq_d = [nc.declare_dram_parameter(f"xq{i}", [128, IC, 512], F16,
                                      isOutput=False) for i in range(4)]
    wqT = nc.declare_dram_parameter("wqT", [128, IC, O], F16, isOutput=False)
    wkT = nc.declare_dram_parameter("wkT", [128, IC, O], F16, isOutput=False)
    wvT = nc.declare_dram_parameter("wvT", [128, IC, O], F16, isOutput=False)
    bqk = nc.declare_dram_parameter("bqk", [128, 4], F32, isOutput=False)
    bv2 = nc.declare_dram_parameter("bv2", [128, 2], F32, isOutput=False)
    mb = nc.declare_dram_parameter("mb", [128, KC], F32, isOutput=False)
    out = nc.declare_dram_parameter("out", [S, O], F32, isOutput=True)

    with tile.TileContext(nc) as tc:
        with tc.tile_pool(name="consts", bufs=1) as consts, \
             tc.tile_pool(name="persist", bufs=1) as persist, \
             tc.tile_pool(name="scp", bufs=1, space="PSUM") as scp, \
             tc.tile_pool(name="pvp", bufs=1, space="PSUM") as pvp, \
             tc.tile_pool(name="fps", bufs=2, space="PSUM") as fps, \
             tc.tile_pool(name="pbp", bufs=2) as pbp, \
             tc.tile_pool(name="ttmp", bufs=2) as ttmp, \
             tc.tile_pool(name="tailp", bufs=2) as tailp:
            ident = consts.tile([128, 128], F16, tag="ident", name="ident")
            make_identity(nc, ident)
            mb_sb = consts.tile([128, KC], F32, tag="mb", name="mb_sb")
            bqk_sb = consts.tile([128, 4], F32, tag="bqk", name="bqk_sb")
            bv2_sb = consts.tile([128, 2], F32, tag="bv2", name="bv2_sb")

            xt = [persist.tile([128, IC, 512], F16, tag=f"xt{i}",
                               name=f"xt{i}") for i in range(4)]
            wq = persist.tile([128, IC, O], F16, tag="wq", name="wqs")
            wk = persist.tile([128, IC, O], F16, tag="wk", name="wks")
            wv = persist.tile([128, IC, O], F16, tag="wv", name="wvs")
            qT = [persist.tile([128, S], F16, tag=f"qT{i}", name=f"qT{i}")
                  for i in range(2)]
            kT = [persist.tile([128, S], F16, tag=f"kT{i}", name=f"kT{i}")
                  for i in range(2)]
            vT = [persist.tile([128, S], F16, tag=f"vT{i}", name=f"vT{i}")
                  for i in range(2)]
            # [key-in-chunk, kc, head-half, d + denominator-ones]
            vS = [persist.tile([128, KC, 2, HD + 1], F16, tag=f"vS{i}",
                               name=f"vS{i}") for i in range(2)]
            for ob in range(2):
                nc.vector.memset(vS[ob][:, :, :, HD:HD + 1], 1.0)

            # ACT-queue: exp-table warmup, consts, v weights, odd x
            # quarters; SP-queue: q/k weights, even x quarters.
            dummy = consts.tile([128, 1], F32, tag="dummy", name="dummy")
            nc.vector.memset(dummy, 0.0)
            nc.scalar.activation(dummy, dummy, EXP)
            nc.scalar.dma_start(out=mb_sb, in_=mb[:, :])
            nc.scalar.dma_start(out=bqk_sb, in_=bqk[:, :])
            nc.scalar.dma_start(out=bv2_sb, in_=bv2[:, :])
            nc.scalar.dma_start(out=wq, in_=wqT[:, :, :])
            nc.sync.dma_start(out=xt[0], in_=xq_d[0][:, :, :])
            nc.scalar.dma_start(out=wv, in_=wvT[:, :, :])
            nc.sync.dma_start(out=wk, in_=wkT[:, :, :])
            nc.scalar.dma_start(out=xt[1], in_=xq_d[1][:, :, :])
            nc.sync.dma_start(out=xt[2], in_=xq_d[2][:, :, :])
            nc.scalar.dma_start(out=xt[3], in_=xq_d[3][:, :, :])

            # clock-ramp warmup in the filler psum slot: keeps the PE busy
            # during the x DMA so the first projection chains run at full
            # clock; retired before any real chain needs the slot
            warm = consts.tile([128, 512], F16, tag="warm", name="warm")
            nc.vector.memset(warm, 0.0)
            for w in range(10):
                wps = fps.tile([128, 512], F32, tag="fp", name=f"warm{w}")
                nc.tensor.matmul(wps, lhsT=warm[:, 0:128], rhs=warm,
                                 start=True, stop=True)

            def proj_qk(wt, ob, dest, bcol, sc):
                ps = fps.tile([128, 512], F32, tag="fp",
                              name=f"pqk{bcol}_{sc}")
                for i in range(IC):
                    nc.tensor.matmul(
                        ps,
                        lhsT=wt[:, i, ob * 128:(ob + 1) * 128],
                        rhs=xt[sc][:, i, :],
                        start=(i == 0), stop=(i == IC - 1))
                nc.vector.tensor_scalar_add(
                    dest[:, sc * 512:(sc + 1) * 512], ps,
                    bqk_sb[:, bcol:bcol + 1])

            def proj_vt(ob, sc):
                ps = fps.tile([128, 512], F32, tag="fp", name=f"pvt{ob}_{sc}")
                for i in range(IC):
                    nc.tensor.matmul(
                        ps,
                        lhsT=wv[:, i, ob * 128:(ob + 1) * 128],
                        rhs=xt[sc][:, i, :],
                        start=(i == 0), stop=(i == IC - 1))
                nc.vector.tensor_scalar_add(
                    vT[ob][:, sc * 512:(sc + 1) * 512], ps,
                    bv2_sb[:, ob:ob + 1])
                for eh in range(2):
                    # tt[p, kc, d] = vT[64*eh + d, 512*sc + 128*kc + p];
                    # XBAR lands in a plain contiguous tile (HW mishandles
                    # scattered out APs), DVE scatters into vS
                    tt = ttmp.tile([128, 4, HD], F16, tag="tt",
                                   name=f"tt{ob}_{sc}_{eh}")
                    nc.sync.dma_start_transpose(
                        out=tt,
                        in_=vT[ob][eh * 64:(eh + 1) * 64,
                                   sc * 512:(sc + 1) * 512])
                    nc.vector.tensor_copy(
                        vS[ob][:, 4 * sc:4 * sc + 4, eh, 0:HD], tt)

            # ---- pre-attention: the bare minimum for block 0 iter 0
            proj_qk(wq, 0, qT[0], 0, 0)
            proj_qk(wk, 0, kT[0], 2, 0)
            proj_vt(0, 0)

            # fillers spread across blocks 0-5, front-loaded only where a
            # deadline forces it (k0 chunk c / vT0 chunk c feed kc 4c..4c+3
            # of every hp0 block; q0 chunk c feeds hp0 block c; q1/k1/vT1
            # feed hp1 = blocks 4-7)
            fill = {}
            fill[(0, 0)] = lambda: proj_qk(wk, 0, kT[0], 2, 1)
            fill[(0, 1)] = lambda: proj_vt(0, 1)
            fill[(0, 4)] = lambda: proj_qk(wk, 0, kT[0], 2, 2)
            fill[(0, 6)] = lambda: proj_vt(0, 2)
            fill[(0, 8)] = lambda: proj_qk(wk, 0, kT[0], 2, 3)
            fill[(0, 10)] = lambda: proj_vt(0, 3)
            fill[(0, 13)] = lambda: proj_qk(wq, 0, qT[0], 0, 1)
            fill[(1, 1)] = lambda: proj_qk(wq, 0, qT[0], 0, 2)
            fill[(1, 6)] = lambda: proj_vt(1, 0)
            fill[(1, 11)] = lambda: proj_qk(wk, 1, kT[1], 3, 0)
            fill[(2, 1)] = lambda: proj_qk(wq, 0, qT[0], 0, 3)
            fill[(2, 6)] = lambda: proj_vt(1, 1)
            fill[(2, 11)] = lambda: proj_qk(wk, 1, kT[1], 3, 1)
            fill[(2, 14)] = lambda: proj_qk(wq, 1, qT[1], 1, 0)
            fill[(3, 1)] = lambda: proj_vt(1, 2)
            fill[(3, 4)] = lambda: proj_qk(wk, 1, kT[1], 3, 2)
            fill[(3, 7)] = lambda: proj_vt(1, 3)
            fill[(3, 10)] = lambda: proj_qk(wk, 1, kT[1], 3, 3)
            fill[(3, 13)] = lambda: proj_qk(wq, 1, qT[1], 1, 1)
            fill[(4, 5)] = lambda: proj_qk(wq, 1, qT[1], 1, 2)
            fill[(5, 5)] = lambda: proj_qk(wq, 1, qT[1], 1, 3)

            # ---- attention blocks
            blocks = [(hp, qc) for hp in range(2) for qc in range(NQ)]
            pending_tail = [None]

            def tail_a(pv, bi):
                ovt = []
                for e in range(2):
                    o_t = tailp.tile([HD + 1, QB], F16, tag=f"ovt{e}",
                                     name=f"ovt{e}_{bi}")
                    nc.vector.tensor_copy(o_t, pv[e])
                    ovt.append(o_t)
                return ovt

            def tail_b(hp, qc, ovt):
                nj = QB // 128
                for e in range(2):
                    hh = 2 * hp + e
                    tr = scp.tile([128, nj, 128], F16, tag=f"sc{e}",
                                  name=f"tr{e}_{hp}{qc}")
                    for jb in range(nj):
                        nc.tensor.transpose(
                            tr[:, jb, 0:HD + 1],
                            ovt[e][:, jb * 128:(jb + 1) * 128],
                            ident[0:HD + 1, 0:HD + 1])
                    rc = tailp.tile([128, nj], F32, tag=f"rc{e}",
                                    name=f"rc{e}_{hp}{qc}")
                    nc.vector.reciprocal(rc, tr[:, :, HD])
                    osb = tailp.tile([128, nj * HD], F32, tag=f"osb{e}",
                                     name=f"osb{e}_{hp}{qc}")
                    for jb in range(nj):
                        nc.vector.tensor_scalar_mul(
                            osb[:, jb * HD:(jb + 1) * HD],
                            tr[:, jb, 0:HD], rc[:, jb:jb + 1])
                    dst = out[qc * QB:(qc + 1) * QB, hh * HD:(hh + 1) * HD]
                    dst = dst.rearrange("(j p) d -> p j d", p=128)
                    nc.sync.dma_start(
                        out=dst,
                        in_=osb.rearrange("p (j d) -> p j d", j=nj))

            for bi, (hp, qc) in enumerate(blocks):
                pv = [pvp.tile([HD + 1, QB], F32, tag=f"pv{e}",
                               name=f"pv{e}_{bi}") for e in range(2)]
                for kc in range(KC):
                    t = scp.tile([128, 2 * QB], F32, tag=f"sc{kc % 2}",
                                 name=f"ts_{bi}_{kc}")
                    for e in range(2):
                        nc.tensor.matmul(
                            t[:, e * QB:(e + 1) * QB],
                            lhsT=kT[hp][e * 64:e * 64 + 64,
                                        kc * 128:(kc + 1) * 128],
                            rhs=qT[hp][e * 64:e * 64 + 64,
                                       qc * QB:(qc + 1) * QB],
                            start=True, stop=True)
                    pb = pbp.tile([128, 2 * QB], F16, tag=f"pb{kc % 2}",
                                  name=f"pb_{bi}_{kc}")
                    nc.scalar.activation(
                        pb, t, EXP, bias=mb_sb[:, kc:kc + 1], scale=0.125)
                    for e in range(2):
                        nc.tensor.matmul(
                            pv[e],
                            lhsT=vS[hp][:, kc, e, :],
                            rhs=pb[:, e * QB:(e + 1) * QB],
                            start=(kc == 0), stop=(kc == KC - 1))
                    if kc == 2 and pending_tail[0] is not None:
                        tail_b(*pending_tail[0])
                        pending_tail[0] = None
                    f = fill.pop((bi, kc), None)
                    if f is not None:
                        f()
                ovt = tail_a(pv, bi)
                pending_tail[0] = (hp, qc, ovt)
            tail_b(*pending_tail[0])
    nc.finalize()
    return nc


_NC_CACHE = None


def _get_nc():
    global _NC_CACHE
    if _NC_CACHE is None:
        _NC_CACHE = build_nc()
    return _NC_CACHE


def _hmaj(a):
    # [1024, N] -> [128, 8, N]: out[p, c, n] = a[128c + p, n]
    n = a.shape[1]
    return np.ascontiguousarray(
        a.reshape(IC, 128, n).transpose(1, 0, 2)).astype(np.float16)


def make_in_maps(inputs, attention_mask, Wq, bq, Wk, bk, Wv, bv):
    x = np.asarray(inputs, dtype=np.float32)
    mask = np.asarray(attention_mask)
    Wq = np.asarray(Wq, dtype=np.float32)
    Wk = np.asarray(Wk, dtype=np.float32)
    Wv = np.asarray(Wv, dtype=np.float32)
    bq = np.asarray(bq, dtype=np.float32)
    bk = np.asarray(bk, dtype=np.float32)
    bv = np.asarray(bv, dtype=np.float32)

    xqb = [[_hmaj(np.ascontiguousarray(x[b].T[:, qt * 512:(qt + 1) * 512]))
            for qt in range(4)] for b in range(B)]
    mbb = [np.ascontiguousarray(
        ((1.0 - mask[b].astype(np.float32)) * NEG).reshape(KC, 128).T)
        for b in range(B)]
    in_maps = []
    for c in range(8):
        b, g = c // G, c % G
        cols = slice(g * O, (g + 1) * O)
        bqs, bks = bq[cols], bk[cols]
        bvs = bv[cols]
        # per-partition v bias for the vT orientation: partition p of
        # vT[ob] is head 2*ob + p//64, feature p%64
        bv2c = np.stack(
            [bvs[ob * 128:(ob + 1) * 128] for ob in range(2)], axis=1)
        im = {f"xq{qt}": xqb[b][qt] for qt in range(4)}
        in_maps.append({
            **im,
            "wqT": _hmaj(np.ascontiguousarray(Wq.T[:, cols])),
            "wkT": _hmaj(np.ascontiguousarray(Wk.T[:, cols])),
            "wvT": _hmaj(np.ascontiguousarray(Wv.T[:, cols])),
            "bqk": np.ascontiguousarray(
                np.stack([bqs[:128], bqs[128:], bks[:128], bks[128:]],
                         axis=1)),
            "bv2": np.ascontiguousarray(bv2c),
            "mb": mbb[b],
        })
    return in_maps


def assemble(results):
    outs = [results[c]["out"] for c in range(8)]
    full = np.stack(
        [np.concatenate(outs[b * G:(b + 1) * G], axis=1) for b in range(B)])
    return np.ascontiguousarray(full.astype(np.float32))


def kernel(**inputs) -> np.ndarray:
    nc = _get_nc()
    in_maps = make_in_maps(**inputs)
    res = run_bass_kernel_spmd(nc, in_maps, core_ids=list(range(8)))
    return assemble(res.results)


# revision 19
# speedup vs baseline: 1.4252x; 1.0240x over previous
"""BertSelfAttention on 8 Trainium2 NeuronCores.

Problem: B=2, S=2048, H=1024, 16 heads x 64. Sharding: batch x head-group
(2 batches x 4 head-groups of 4 heads = 8 cores). Each core computes
q/k/v projections for its 4 heads and full attention over them.

Schedule (v6): ACT must stream 128 exps x ~1.1us back-to-back while the
PE fits attention matmuls (110us) plus all projections (41us) around it,
and the chip halves the tensor clock after ~150us of dense PE activity —
so the design minimizes total PE-busy and keeps the exp stream dense:
  - q-blocks of 512: per kc one merged [128,1024] score tile (both
    heads; exp bias only depends on key position), ONE exp, 2 score +
    2 pv matmuls; psum = 2 score slots (kc-parity) + 2 pv accumulators
    + a dedicated 2-bank slot for projection chains, so projection
    fillers never sit on the score WAR chain
  - V is projected in transposed orientation (vT[d, s], N=512 matmuls
    like q/k) and moved to the [key, d] layout the PV matmul needs by
    XBAR DMA transposes (64-row slabs) — off the PE entirely; the
    denominator ones-column is memset once
  - x is DMA'd as four contiguous s-quarter tensors split across the
    SP and ACT DGE queues; only q0/k0/vT0 chunk-0 chains run before
    attention starts (~10us), every other projection chain is a filler
    inside the attention loops with explicit deadlines
  - block tails (pv copy, fp16 PE transpose, normalize, store) are
    deferred past the next block's second iteration
"""

import sys

sys.path.insert(0, "/opt/trn_rl_repo")

import numpy as np

import concourse.bass as bass
import concourse.tile as tile
from concourse.masks import make_identity
from concourse import bacc, mybir
from concourse.bass_utils import run_bass_kernel_spmd

F32 = mybir.dt.float32
F16 = mybir.dt.float16
EXP = mybir.ActivationFunctionType.Exp

B, S, H = 2, 2048, 1024
NH, HD = 16, 64
G = 4                 # head-groups (cores per batch)
NHL = NH // G         # heads per core
O = NHL * HD          # 256 output features per core
IC = H // 128         # 8 contraction chunks
KC = S // 128         # 16 key chunks
QB = 512              # q block size
NQ = S // QB          # 4 q blocks per head pair
NEG = -1.0e30


def build_nc():
    nc = bacc.Bacc(None, target_bir_lowering=False)
    xq_d = [nc.declare_dram_parameter(f"xq{i}", [128, IC, 512], F16,
                                      isOutput=False) for i in range(4)]
    wqT = nc.declare_dram_parameter("wqT", [128, IC, O], F16, isOutput=False)
    wkT = nc.declare_dram_parameter("wkT", [128, IC, O], F16, isOutput=False)
    wvT = nc.declare_dram_parameter("wvT", [128, IC, O], F16, isOutput=False)
    bqk = nc.declare_dram_parameter("bqk", [128, 4], F32, isOutput=False)
    bv2 = nc.declare_dram_parameter("bv2", [128, 2], F32, isOutput=False)
    mb = nc.declare_dram_parameter("mb", [128, KC], F32, isOutput=False)
    out = nc.declare_dram_parameter("out", [S, O], F32, isOutput=True)

    with tile.TileContext(nc) as tc:
        with tc.tile_pool(name="consts", bufs=1) as consts, \
             tc.tile_pool(name="persist", bufs=1) as persist, \
             tc.tile_pool(name="scp", bufs=1, space="PSUM") as scp, \
             tc.tile_pool(name="pvp", bufs=1, space="PSUM") as pvp, \
             tc.tile_pool(name="fps", bufs=2, space="PSUM") as fps, \
             tc.tile_pool(name="pbp", bufs=2) as pbp, \
             tc.tile_pool(name="ttmp", bufs=2) as ttmp, \
             tc.tile_pool(name="tailp", bufs=2) as tailp:
            ident = consts.tile([128, 128], F16, tag="ident", name="ident")
            make_identity(nc, ident)
            mb_sb = consts.tile([128, KC], F32, tag="mb", name="mb_sb")
            bqk_sb = consts.tile([128, 4], F32, tag="bqk", name="bqk_sb")
            bv2_sb = consts.tile([128, 2], F32, tag="bv2", name="bv2_sb")

            xt = [persist.tile([128, IC, 512], F16, tag=f"xt{i}",
                               name=f"xt{i}") for i in range(4)]
            wq = persist.tile([128, IC, O], F16, tag="wq", name="wqs")
            wk = persist.tile([128, IC, O], F16, tag="wk", name="wks")
            wv = persist.tile([128, IC, O], F16, tag="wv", name="wvs")
            qT = [persist.tile([128, S], F16, tag=f"qT{i}", name=f"qT{i}")
                  for i in range(2)]
            kT = [persist.tile([128, S], F16, tag=f"kT{i}", name=f"kT{i}")
                  for i in range(2)]
            vT = [persist.tile([128, S], F16, tag=f"vT{i}", name=f"vT{i}")
                  for i in range(2)]
            # [key-in-chunk, kc, head-half, d + denominator-ones]
            vS = [persist.tile([128, KC, 2, HD + 1], F16, tag=f"vS{i}",
                               name=f"vS{i}") for i in range(2)]
            for ob in range(2):
                nc.vector.memset(vS[ob][:, :, :, HD:HD + 1], 1.0)

            # ACT-queue: exp-table warmup, consts, v weights, odd x
            # quarters; SP-queue: q/k weights, even x quarters.
            dummy = consts.tile([128, 1], F32, tag="dummy", name="dummy")
            nc.vector.memset(dummy, 0.0)
            nc.scalar.activation(dummy, dummy, EXP)
            nc.scalar.dma_start(out=mb_sb, in_=mb[:, :])
            nc.scalar.dma_start(out=bqk_sb, in_=bqk[:, :])
            nc.scalar.dma_start(out=bv2_sb, in_=bv2[:, :])
            nc.scalar.dma_start(out=wq, in_=wqT[:, :, :])
            nc.sync.dma_start(out=wk, in_=wkT[:, :, :])
            nc.scalar.dma_start(out=wv, in_=wvT[:, :, :])
            nc.sync.dma_start(out=xt[0], in_=xq_d[0][:, :, :])
            nc.scalar.dma_start(out=xt[1], in_=xq_d[1][:, :, :])
            nc.sync.dma_start(out=xt[2], in_=xq_d[2][:, :, :])
            nc.scalar.dma_start(out=xt[3], in_=xq_d[3][:, :, :])

            # clock-ramp warmup in the filler psum slot: keeps the PE busy
            # through the input-DMA window so the first projection chains
            # run at full clock; retired before any real chain needs psum
            warm = consts.tile([128, 512], F16, tag="warm", name="warm")
            nc.vector.memset(warm, 0.0)
            for w in range(16):
                wps = fps.tile([128, 512], F32, tag="fp", name=f"warm{w}")
                nc.tensor.matmul(wps, lhsT=warm[:, 0:128], rhs=warm,
                                 start=True, stop=True)

            def proj_qk(wt, ob, dest, bcol, sc):
                ps = fps.tile([128, 512], F32, tag="fp",
                              name=f"pqk{bcol}_{sc}")
                for i in range(IC):
                    nc.tensor.matmul(
                        ps,
                        lhsT=wt[:, i, ob * 128:(ob + 1) * 128],
                        rhs=xt[sc][:, i, :],
                        start=(i == 0), stop=(i == IC - 1))
                nc.vector.tensor_scalar_add(
                    dest[:, sc * 512:(sc + 1) * 512], ps,
                    bqk_sb[:, bcol:bcol + 1])

            def proj_vt(ob, sc):
                ps = fps.tile([128, 512], F32, tag="fp", name=f"pvt{ob}_{sc}")
                for i in range(IC):
                    nc.tensor.matmul(
                        ps,
                        lhsT=wv[:, i, ob * 128:(ob + 1) * 128],
                        rhs=xt[sc][:, i, :],
                        start=(i == 0), stop=(i == IC - 1))
                nc.vector.tensor_scalar_add(
                    vT[ob][:, sc * 512:(sc + 1) * 512], ps,
                    bv2_sb[:, ob:ob + 1])
                for eh in range(2):
                    # tt[p, kc, d] = vT[64*eh + d, 512*sc + 128*kc + p];
                    # XBAR lands in a plain contiguous tile (HW mishandles
                    # scattered out APs), DVE scatters into vS
                    tt = ttmp.tile([128, 4, HD], F16, tag="tt",
                                   name=f"tt{ob}_{sc}_{eh}")
                    nc.sync.dma_start_transpose(
                        out=tt,
                        in_=vT[ob][eh * 64:(eh + 1) * 64,
                                   sc * 512:(sc + 1) * 512])
                    nc.vector.tensor_copy(
                        vS[ob][:, 4 * sc:4 * sc + 4, eh, 0:HD], tt)

            # ---- pre-attention: the bare minimum for block 0 iter 0
            proj_qk(wq, 0, qT[0], 0, 0)
            proj_qk(wk, 0, kT[0], 2, 0)
            proj_vt(0, 0)

            # fillers spread across blocks 0-5, front-loaded only where a
            # deadline forces it (k0 chunk c / vT0 chunk c feed kc 4c..4c+3
            # of every hp0 block; q0 chunk c feeds hp0 block c; q1/k1/vT1
            # feed hp1 = blocks 4-7)
            fill = {}
            fill[(0, 0)] = lambda: proj_qk(wk, 0, kT[0], 2, 1)
            fill[(0, 1)] = lambda: proj_vt(0, 1)
            fill[(0, 4)] = lambda: proj_qk(wk, 0, kT[0], 2, 2)
            fill[(0, 6)] = lambda: proj_vt(0, 2)
            fill[(0, 8)] = lambda: proj_qk(wk, 0, kT[0], 2, 3)
            fill[(0, 10)] = lambda: proj_vt(0, 3)
            fill[(0, 13)] = lambda: proj_qk(wq, 0, qT[0], 0, 1)
            fill[(1, 1)] = lambda: proj_qk(wq, 0, qT[0], 0, 2)
            fill[(1, 6)] = lambda: proj_vt(1, 0)
            fill[(1, 11)] = lambda: proj_qk(wk, 1, kT[1], 3, 0)
            fill[(2, 1)] = lambda: proj_qk(wq, 0, qT[0], 0, 3)
            fill[(2, 6)] = lambda: proj_vt(1, 1)
            fill[(2, 11)] = lambda: proj_qk(wk, 1, kT[1], 3, 1)
            fill[(2, 14)] = lambda: proj_qk(wq, 1, qT[1], 1, 0)
            fill[(3, 1)] = lambda: proj_vt(1, 2)
            fill[(3, 4)] = lambda: proj_qk(wk, 1, kT[1], 3, 2)
            fill[(3, 7)] = lambda: proj_vt(1, 3)
            fill[(3, 10)] = lambda: proj_qk(wk, 1, kT[1], 3, 3)
            fill[(3, 13)] = lambda: proj_qk(wq, 1, qT[1], 1, 1)
            fill[(4, 5)] = lambda: proj_qk(wq, 1, qT[1], 1, 2)
            fill[(5, 5)] = lambda: proj_qk(wq, 1, qT[1], 1, 3)

            # ---- attention blocks
            blocks = [(hp, qc) for hp in range(2) for qc in range(NQ)]
            pending_tail = [None]

            def tail_a(pv, bi):
                ovt = []
                for e in range(2):
                    o_t = tailp.tile([HD + 1, QB], F16, tag=f"ovt{e}",
                                     name=f"ovt{e}_{bi}")
                    nc.vector.tensor_copy(o_t, pv[e])
                    ovt.append(o_t)
                return ovt

            def tail_b(hp, qc, ovt):
                nj = QB // 128
                for e in range(2):
                    hh = 2 * hp + e
                    tr = fps.tile([128, nj, 128], F16, tag="fp",
                                  name=f"tr{e}_{hp}{qc}")
                    for jb in range(nj):
                        nc.tensor.transpose(
                            tr[:, jb, 0:HD + 1],
                            ovt[e][:, jb * 128:(jb + 1) * 128],
                            ident[0:HD + 1, 0:HD + 1])
                    rc = tailp.tile([128, nj], F32, tag=f"rc{e}",
                                    name=f"rc{e}_{hp}{qc}")
                    nc.vector.reciprocal(rc, tr[:, :, HD])
                    osb = tailp.tile([128, nj * HD], F32, tag=f"osb{e}",
                                     name=f"osb{e}_{hp}{qc}")
                    for jb in range(nj):
                        nc.vector.tensor_scalar_mul(
                            osb[:, jb * HD:(jb + 1) * HD],
                            tr[:, jb, 0:HD], rc[:, jb:jb + 1])
                    dst = out[qc * QB:(qc + 1) * QB, hh * HD:(hh + 1) * HD]
                    dst = dst.rearrange("(j p) d -> p j d", p=128)
                    nc.sync.dma_start(
                        out=dst,
                        in_=osb.rearrange("p (j d) -> p j d", j=nj))

            for bi, (hp, qc) in enumerate(blocks):
                pv = [pvp.tile([HD + 1, QB], F32, tag=f"pv{e}",
                               name=f"pv{e}_{bi}") for e in range(2)]
                for kc in range(KC):
                    t = scp.tile([128, 2 * QB], F32, tag=f"sc{kc % 2}",
                                 name=f"ts_{bi}_{kc}")
                    for e in range(2):
                        nc.tensor.matmul(
                            t[:, e * QB:(e + 1) * QB],
                            lhsT=kT[hp][e * 64:e * 64 + 64,
                                        kc * 128:(kc + 1) * 128],
                            rhs=qT[hp][e * 64:e * 64 + 64,
                                       qc * QB:(qc + 1) * QB],
                            start=True, stop=True)
                    pb = pbp.tile([128, 2 * QB], F16, tag=f"pb{kc % 2}",
                                  name=f"pb_{bi}_{kc}")
                    nc.scalar.activation(
                        pb, t, EXP, bias=mb_sb[:, kc:kc + 1], scale=0.125)
                    for e in range(2):
                        nc.tensor.matmul(
                            pv[e],
                            lhsT=vS[hp][:, kc, e, :],
                            rhs=pb[:, e * QB:(e + 1) * QB],
                            start=(kc == 0), stop=(kc == KC - 1))
                    if kc == 2 and pending_tail[0] is not None:
                        tail_b(*pending_tail[0])
                        pending_tail[0] = None
                    f = fill.pop((bi, kc), None)
                    if f is not None:
                        f()
                ovt = tail_a(pv, bi)
                pending_tail[0] = (hp, qc, ovt)
            tail_b(*pending_tail[0])
    nc.finalize()
    return nc


_NC_CACHE = None


def _get_nc():
    global _NC_CACHE
    if _NC_CACHE is None:
        _NC_CACHE = build_nc()
    return _NC_CACHE


def _hmaj(a):
    # [1024, N] -> [128, 8, N]: out[p, c, n] = a[128c + p, n]
    n = a.shape[1]
    return np.ascontiguousarray(
        a.reshape(IC, 128, n).transpose(1, 0, 2)).astype(np.float16)


def make_in_maps(inputs, attention_mask, Wq, bq, Wk, bk, Wv, bv):
    x = np.asarray(inputs, dtype=np.float32)
    mask = np.asarray(attention_mask)
    Wq = np.asarray(Wq, dtype=np.float32)
    Wk = np.asarray(Wk, dtype=np.float32)
    Wv = np.asarray(Wv, dtype=np.float32)
    bq = np.asarray(bq, dtype=np.float32)
    bk = np.asarray(bk, dtype=np.float32)
    bv = np.asarray(bv, dtype=np.float32)

    xqb = [[_hmaj(np.ascontiguousarray(x[b].T[:, qt * 512:(qt + 1) * 512]))
            for qt in range(4)] for b in range(B)]
    mbb = [np.ascontiguousarray(
        ((1.0 - mask[b].astype(np.float32)) * NEG).reshape(KC, 128).T)
        for b in range(B)]
    in_maps = []
    for c in range(8):
        b, g = c // G, c % G
        cols = slice(g * O, (g + 1) * O)
        bqs, bks = bq[cols], bk[cols]
        bvs = bv[cols]
        # per-partition v bias for the vT orientation: partition p of
        # vT[ob] is head 2*ob + p//64, feature p%64
        bv2c = np.stack(
            [bvs[ob * 128:(ob + 1) * 128] for ob in range(2)], axis=1)
        im = {f"xq{qt}": xqb[b][qt] for qt in range(4)}
        in_maps.append({
            **im,
            "wqT": _hmaj(np.ascontiguousarray(Wq.T[:, cols])),
            "wkT": _hmaj(np.ascontiguousarray(Wk.T[:, cols])),
            "wvT": _hmaj(np.ascontiguousarray(Wv.T[:, cols])),
            "bqk": np.ascontiguousarray(
                np.stack([bqs[:128], bqs[128:], bks[:128], bks[128:]],
                         axis=1)),
            "bv2": np.ascontiguousarray(bv2c),
            "mb": mbb[b],
        })
    return in_maps


def assemble(results):
    outs = [results[c]["out"] for c in range(8)]
    full = np.stack(
        [np.concatenate(outs[b * G:(b + 1) * G], axis=1) for b in range(B)])
    return np.ascontiguousarray(full.astype(np.float32))


def kernel(**inputs) -> np.ndarray:
    nc = _get_nc()
    in_maps = make_in_maps(**inputs)
    res = run_bass_kernel_spmd(nc, in_maps, core_ids=list(range(8)))
    return assemble(res.results)
